# revision 5
# baseline (speedup 1.0000x reference)
"""TRN2 Bass kernel for nn_MultiHeadAttention_26156350832790.

Multi-head attention: B=1, S=2048, D=2048, H=16 heads (dk=128), causal mask,
fp32 I/O.  Sharded tensor-parallel over 8 NeuronCores: 2 heads per core.

Per-core dataflow (all matmuls in float32r at full PE rate):
  phase 1: Q^T/K^T [dk, S] and V [S, dk] projections, streaming x^T blocks
  phase 2: flash-style attention per (head, 512-wide q-chunk), scores kept
           TRANSPOSED [k, q] so softmax sums come from a ones-matmul and the
           PV matmul needs no P transpose; normalization applied to the
           accumulated output via a broadcast reciprocal multiply
  phase 3: O-projection (contraction over the core's 256 head-dims),
           producing a partial [S, D] summed across cores on the host

Host side: x is pre-transposed, weights pre-tiled into SBUF-friendly
layouts, causal masks precomputed; bq/bk applied in-kernel at Q/K
evacuation, bv/bo folded into a host-side row-vector add (softmax rows sum
to 1, so P @ (V + bv) == P @ V + bv exactly).
"""

import math
import os
import sys

if "/opt/trn_rl_repo" not in sys.path:
    sys.path.insert(0, "/opt/trn_rl_repo")

import numpy as np

import concourse.bacc as bacc
import concourse.tile as tile
from concourse import mybir
from concourse.bass_utils import run_bass_kernel_spmd

P = 128          # partitions
S = 2048         # sequence
D = 2048         # model dim
NT = 16          # 128-row tiles in S or D
HPC = 2          # heads per core
DK = 128         # head dim
NB = 8           # phase-1 s-blocks
BW = 256         # phase-1 block width (s columns)
C = 4            # attention q-chunks
CW = 512         # chunk width
N_CORES = 8
SCALE = 1.0 / math.sqrt(DK)
NEG = -1.0e30

F = mybir.dt.float32
R = mybir.dt.float32r

_NC = None
last_exec_time_ns = None


def build():
    nc = bacc.Bacc(None)

    xT = nc.dram_tensor("xT", [D, S], R, kind="ExternalInput")
    wq = nc.dram_tensor("wq", [P, NT * 2 * DK], R, kind="ExternalInput")
    wk = nc.dram_tensor("wk", [P, NT * 2 * DK], R, kind="ExternalInput")
    wv = nc.dram_tensor("wv", [P, NT * 2 * DK], R, kind="ExternalInput")
    wo = nc.dram_tensor("wo", [P, HPC * D], R, kind="ExternalInput")
    bqk = nc.dram_tensor("bqk", [P, 4], F, kind="ExternalInput")
    masks = nc.dram_tensor("masks", [P, 4 * CW], F, kind="ExternalInput")
    onesc = nc.dram_tensor("onesc", [P, 1], R, kind="ExternalInput")
    onesr = nc.dram_tensor("onesr", [1, P], R, kind="ExternalInput")
    out = nc.dram_tensor("out", [S, D], F, kind="ExternalOutput")

    Exp = mybir.ActivationFunctionType.Exp

    with tile.TileContext(nc) as tc:
        with (
            tc.tile_pool(name="consts", bufs=1) as consts,
            tc.tile_pool(name="persist", bufs=1) as persist,
        ):
            wq_sb = consts.tile([P, NT * 2 * DK], R)
            wk_sb = consts.tile([P, NT * 2 * DK], R)
            wv_sb = consts.tile([P, NT * 2 * DK], R)
            wo_sb = consts.tile([P, HPC * D], R)
            bqk_sb = consts.tile([P, 4], F)
            masks_sb = consts.tile([P, 4 * CW], F)
            ones_col = consts.tile([P, 1], R)
            ones_row = consts.tile([1, P], R)
            nc.sync.dma_start(out=wq_sb, in_=wq[:])
            nc.sync.dma_start(out=wk_sb, in_=wk[:])
            nc.sync.dma_start(out=wv_sb, in_=wv[:])
            nc.sync.dma_start(out=wo_sb, in_=wo[:])
            nc.sync.dma_start(out=bqk_sb, in_=bqk[:])
            nc.sync.dma_start(out=masks_sb, in_=masks[:])
            nc.sync.dma_start(out=ones_col, in_=onesc[:])
            nc.sync.dma_start(out=ones_row, in_=onesr[:])

            # persistent activations
            qt_sb = persist.tile([P, HPC * S], R)      # Q^T per head [dk, S]
            kt_sb = persist.tile([P, HPC * S], R)      # K^T per head [dk, S]
            v_sb = persist.tile([P, NT * 2 * DK], R)   # V s-tiles [k, 2*dk]
            attnT_sb = persist.tile([P, HPC * S], R)   # attn^T per head [dk, S]

            # ---- phase 1: QKV projections, streaming x^T in 256-col blocks
            xT_tiled = xT.rearrange("(t p) s -> p t s", p=P)
            with (
                tc.tile_pool(name="xin", bufs=2) as xin,
                tc.tile_pool(name="p1ps", bufs=2, space="PSUM") as p1ps,
                tc.tile_pool(name="p1ev", bufs=3) as _unused_p1ev,  # noqa: F841
            ):
                for b in range(NB):
                    xt = xin.tile([P, NT, BW], R, name="xt")
                    nc.sync.dma_start(
                        out=xt, in_=xT_tiled[:, :, b * BW : (b + 1) * BW]
                    )
                    for h in range(HPC):
                        qps = p1ps.tile([P, BW], F, name="qps")
                        kps = p1ps.tile([P, BW], F, name="kps")
                        for t in range(NT):
                            nc.tensor.matmul(
                                qps[:],
                                wq_sb[:, t * 2 * DK + h * DK : t * 2 * DK + (h + 1) * DK],
                                xt[:, t, :],
                                start=(t == 0),
                                stop=(t == NT - 1),
                            )
                        for t in range(NT):
                            nc.tensor.matmul(
                                kps[:],
                                wk_sb[:, t * 2 * DK + h * DK : t * 2 * DK + (h + 1) * DK],
                                xt[:, t, :],
                                start=(t == 0),
                                stop=(t == NT - 1),
                            )
                        nc.scalar.add(
                            qt_sb[:, h * S + b * BW : h * S + (b + 1) * BW],
                            qps[:],
                            bqk_sb[:, h : h + 1],
                        )
                        nc.scalar.add(
                            kt_sb[:, h * S + b * BW : h * S + (b + 1) * BW],
                            kps[:],
                            bqk_sb[:, 2 + h : 3 + h],
                        )
                    for u2 in range(2):
                        vps = p1ps.tile([P, BW], F, name="vps")
                        for t in range(NT):
                            nc.tensor.matmul(
                                vps[:],
                                xt[:, t, u2 * DK : (u2 + 1) * DK],
                                wv_sb[:, t * 2 * DK : (t + 1) * 2 * DK],
                                start=(t == 0),
                                stop=(t == NT - 1),
                            )
                        u = 2 * b + u2
                        nc.scalar.copy(
                            v_sb[:, u * 2 * DK : (u + 1) * 2 * DK], vps[:]
                        )

            # ---- phase 2: causal attention, scores transposed [k, q]
            with (
                tc.tile_pool(name="scps", bufs=2, space="PSUM") as scps,
                tc.tile_pool(name="smps", bufs=2, space="PSUM") as smps,
                tc.tile_pool(name="ops", bufs=2, space="PSUM") as ops,
                tc.tile_pool(name="bps", bufs=1, space="PSUM") as bps,
                tc.tile_pool(name="ptp", bufs=4) as ptp,
                tc.tile_pool(name="rsp", bufs=2) as rsp,
                tc.tile_pool(name="bcp", bufs=2) as bcp,
            ):
                for h in range(HPC):
                    for c in range(C):
                        jmax = 4 * c + 3
                        sum_ps = smps.tile([1, CW], F, name="sum_ps")
                        o_ps = ops.tile([P, CW], F, name="o_ps")
                        for j in range(jmax + 1):
                            sc = scps.tile([P, CW], F, name="sc")
                            nc.tensor.matmul(
                                sc[:],
                                kt_sb[:, h * S + j * P : h * S + (j + 1) * P],
                                qt_sb[:, h * S + c * CW : h * S + (c + 1) * CW],
                                start=True,
                                stop=True,
                            )
                            if j >= 4 * c:
                                t = j - 4 * c
                                nc.vector.tensor_add(
                                    sc[:], sc[:], masks_sb[:, t * CW : (t + 1) * CW]
                                )
                            pt = ptp.tile([P, CW], R, name="pt")
                            nc.scalar.activation(pt[:], sc[:], Exp, scale=SCALE)
                            nc.tensor.matmul(
                                sum_ps[:],
                                ones_col[:],
                                pt[:],
                                start=(j == 0),
                                stop=(j == jmax),
                            )
                            nc.tensor.matmul(
                                o_ps[:],
                                v_sb[:, j * 2 * DK + h * DK : j * 2 * DK + (h + 1) * DK],
                                pt[:],
                                start=(j == 0),
                                stop=(j == jmax),
                            )
                        rs = rsp.tile([1, CW], R, name="rs")
                        with nc.allow_low_precision(reason="fp32r feed to bcast matmul"):
                            nc.vector.reciprocal(rs[:], sum_ps[:])
                        b_ps = bps.tile([P, CW], F, name="b_ps")
                        nc.tensor.matmul(
                            b_ps[:], ones_row[:], rs[:], start=True, stop=True
                        )
                        bc = bcp.tile([P, CW], F, name="bc")
                        nc.scalar.copy(bc[:], b_ps[:])
                        nc.vector.tensor_mul(
                            attnT_sb[:, h * S + c * CW : h * S + (c + 1) * CW],
                            o_ps[:],
                            bc[:],
                        )

            # ---- phase 3: O-projection, partial over this core's heads
            with (
                tc.tile_pool(name="o3ps", bufs=4, space="PSUM") as o3ps,
                tc.tile_pool(name="outp", bufs=4) as outp,
            ):
                for u in range(NT):
                    for e in range(C):
                        o3 = o3ps.tile([P, CW], F, name="o3")
                        for h in range(HPC):
                            nc.tensor.matmul(
                                o3[:],
                                attnT_sb[:, h * S + u * P : h * S + (u + 1) * P],
                                wo_sb[:, h * D + e * CW : h * D + (e + 1) * CW],
                                start=(h == 0),
                                stop=(h == HPC - 1),
                            )
                        ot = outp.tile([P, CW], F, name="ot")
                        if (u * C + e) % 2 == 0:
                            nc.scalar.copy(ot[:], o3[:])
                        else:
                            nc.vector.tensor_copy(ot[:], o3[:])
                        nc.sync.dma_start(
                            out=out[u * P : (u + 1) * P, e * CW : (e + 1) * CW],
                            in_=ot[:],
                        )

    nc.compile()
    return nc


def _tile_weight_cols(w_slice: np.ndarray) -> np.ndarray:
    """[2048, 256] -> [128, 16*256] with block t = rows [128t, 128t+128)."""
    return np.ascontiguousarray(
        w_slice.reshape(NT, P, 2 * DK).transpose(1, 0, 2).reshape(P, NT * 2 * DK)
    )


def _make_masks() -> np.ndarray:
    m = np.zeros((P, 4 * CW), dtype=np.float32)
    p = np.arange(P)[:, None]
    f = np.arange(CW)[None, :]
    for t in range(4):
        m[:, t * CW : (t + 1) * CW] = np.where(t * P + p <= f, 0.0, NEG)
    return m


def kernel(x, Wq, bq, Wk, bk, Wv, bv, Wo, bo):
    global _NC, last_exec_time_ns

    x = np.asarray(x, dtype=np.float32)
    Wq = np.asarray(Wq, dtype=np.float32)
    Wk = np.asarray(Wk, dtype=np.float32)
    Wv = np.asarray(Wv, dtype=np.float32)
    Wo = np.asarray(Wo, dtype=np.float32)
    bq = np.asarray(bq, dtype=np.float32)
    bk = np.asarray(bk, dtype=np.float32)
    bv = np.asarray(bv, dtype=np.float32)
    bo = np.asarray(bo, dtype=np.float32)

    if _NC is None:
        _NC = build()

    xT = np.ascontiguousarray(x[0].T)
    masks = _make_masks()

    in_maps = []
    for i in range(N_CORES):
        cs = slice(2 * DK * i, 2 * DK * (i + 1))
        bqk_i = np.stack(
            [
                bq[2 * DK * i : 2 * DK * i + DK],
                bq[2 * DK * i + DK : 2 * DK * (i + 1)],
                bk[2 * DK * i : 2 * DK * i + DK],
                bk[2 * DK * i + DK : 2 * DK * (i + 1)],
            ],
            axis=1,
        ).astype(np.float32)
        wo_i = np.ascontiguousarray(
            Wo[cs, :].reshape(HPC, P, D).transpose(1, 0, 2).reshape(P, HPC * D)
        )
        in_maps.append(
            {
                "xT": xT,
                "wq": _tile_weight_cols(Wq[:, cs]),
                "wk": _tile_weight_cols(Wk[:, cs]),
                "wv": _tile_weight_cols(Wv[:, cs]),
                "wo": wo_i,
                "bqk": bqk_i,
                "masks": masks,
                "onesc": np.ones((P, 1), np.float32),
                "onesr": np.ones((1, P), np.float32),
            }
        )

    trace = bool(int(os.environ.get("BASS_TRACE", "0") or "0"))
    if trace:
        try:
            import ntff_shim

            ntff_shim.install()
        except Exception:
            pass

    res = run_bass_kernel_spmd(
        _NC, in_maps, core_ids=list(range(N_CORES)), trace=trace
    )
    last_exec_time_ns = res.exec_time_ns

    acc = np.zeros((S, D), dtype=np.float64)
    for r_ in res.results:
        acc += r_["out"].astype(np.float64)
    # bv/bo fold: softmax rows sum to 1 => attn @ (V+bv) @ Wo + bo adds bv@Wo + bo
    acc += bv.astype(np.float64) @ Wo.astype(np.float64) + bo.astype(np.float64)
    return acc.astype(np.float32).reshape(1, S, D)


# revision 7
# speedup vs baseline: 1.0656x; 1.0656x over previous
"""TRN2 Bass kernel for nn_MultiHeadAttention_26156350832790.

Multi-head attention: B=1, S=2048, D=2048, H=16 heads (dk=128), causal mask,
fp32 I/O.  Sharded tensor-parallel over 8 NeuronCores: 2 heads per core.

Per-core dataflow (all matmuls in float32r at full PE rate):
  phase 1: Q^T/K^T [dk, S] and V [S, dk] projections, streaming x^T in
           512-column blocks; Q/K (N=512) interleaved with V (N=256) so
           LDWEIGHTS pipelines under the streams
  phase 2: flash-style attention per (head, 512-wide q-chunk), scores kept
           TRANSPOSED [k, q] so softmax sums come from a ones-matmul and the
           PV matmul needs no P transpose; causal handled by slicing the
           q-range per diagonal k-tile plus one [128,128] triangle mask;
           normalization applied to the accumulated output via a broadcast
           reciprocal multiply
  phase 3: O-projection (contraction over the core's 256 head-dims),
           producing a partial [S, D] summed across cores on the host

Host side: x is pre-transposed, weights pre-tiled into SBUF-friendly
layouts; bq/bk applied in-kernel at Q/K evacuation, bv/bo folded into a
host-side row-vector add (softmax rows sum to 1, so P @ (V + bv) ==
P @ V + bv exactly).
"""

import math
import os
import sys

if "/opt/trn_rl_repo" not in sys.path:
    sys.path.insert(0, "/opt/trn_rl_repo")

import numpy as np

import concourse.bacc as bacc
import concourse.tile as tile
from concourse import mybir
from concourse.bass_utils import run_bass_kernel_spmd

P = 128          # partitions
S = 2048         # sequence
D = 2048         # model dim
NT = 16          # 128-row tiles in S or D
HPC = 2          # heads per core
DK = 128         # head dim
C = 4            # 512-wide chunks
CW = 512         # chunk width
N_CORES = 8
SCALE = 1.0 / math.sqrt(DK)
NEG = -1.0e30

F = mybir.dt.float32
R = mybir.dt.float32r

_NC = None
last_exec_time_ns = None
_last_in_maps = None


def build():
    nc = bacc.Bacc(None)

    xT = nc.dram_tensor("xT", [D, S], R, kind="ExternalInput")
    wq = nc.dram_tensor("wq", [P, NT * 2 * DK], R, kind="ExternalInput")
    wk = nc.dram_tensor("wk", [P, NT * 2 * DK], R, kind="ExternalInput")
    wv = nc.dram_tensor("wv", [P, NT * 2 * DK], R, kind="ExternalInput")
    wo = nc.dram_tensor("wo", [P, HPC * D], R, kind="ExternalInput")
    bqk = nc.dram_tensor("bqk", [P, 4], F, kind="ExternalInput")
    masks = nc.dram_tensor("masks", [P, P], F, kind="ExternalInput")
    onesc = nc.dram_tensor("onesc", [P, 1], R, kind="ExternalInput")
    onesr = nc.dram_tensor("onesr", [1, P], R, kind="ExternalInput")
    out = nc.dram_tensor("out", [S, D], F, kind="ExternalOutput")

    Exp = mybir.ActivationFunctionType.Exp

    with tile.TileContext(nc) as tc:
        with (
            tc.tile_pool(name="consts", bufs=1) as consts,
            tc.tile_pool(name="persist", bufs=1) as persist,
        ):
            bqk_sb = consts.tile([P, 4], F)
            mask_sb = consts.tile([P, P], F)
            ones_col = consts.tile([P, 1], R)
            ones_row = consts.tile([1, P], R)
            nc.sync.dma_start(out=bqk_sb, in_=bqk[:])
            nc.sync.dma_start(out=mask_sb, in_=masks[:])
            nc.sync.dma_start(out=ones_col, in_=onesc[:])
            nc.sync.dma_start(out=ones_row, in_=onesr[:])

            # persistent activations
            qt_sb = persist.tile([P, HPC * S], R)      # Q^T per head [dk, S]
            kt_sb = persist.tile([P, HPC * S], R)      # K^T per head [dk, S]
            v_sb = persist.tile([P, NT * 2 * DK], R)   # V s-tiles [k, 2*dk]
            attnT_sb = persist.tile([P, HPC * S], R)   # attn^T per head [dk, S]

            # ---- phase 1: QKV projections, streaming x^T in 512-col blocks
            xT_tiled = xT.rearrange("(t p) s -> p t s", p=P)
            with (
                tc.tile_pool(name="wqkv", bufs=1) as wqkv,
                tc.tile_pool(name="xin", bufs=2) as xin,
                tc.tile_pool(name="p1ps", bufs=2, space="PSUM") as p1ps,
            ):
                wq_sb = wqkv.tile([P, NT * 2 * DK], R)
                wk_sb = wqkv.tile([P, NT * 2 * DK], R)
                wv_sb = wqkv.tile([P, NT * 2 * DK], R)
                nc.sync.dma_start(out=wq_sb, in_=wq[:])
                nc.sync.dma_start(out=wk_sb, in_=wk[:])
                nc.sync.dma_start(out=wv_sb, in_=wv[:])

                for b in range(C):
                    xt = xin.tile([P, NT, CW], R, name="xt")
                    nc.sync.dma_start(
                        out=xt, in_=xT_tiled[:, :, b * CW : (b + 1) * CW]
                    )
                    for h in range(HPC):
                        qps = p1ps.tile([P, CW], F, name="qps")
                        kps = p1ps.tile([P, CW], F, name="kps")
                        vps0 = p1ps.tile([P, 2 * DK], F, name="vps0")
                        vps1 = p1ps.tile([P, 2 * DK], F, name="vps1")
                        i0 = 2 * h
                        for t in range(NT):
                            st = t == 0
                            sp = t == NT - 1
                            nc.tensor.matmul(
                                qps[:],
                                wq_sb[:, t * 2 * DK + h * DK : t * 2 * DK + (h + 1) * DK],
                                xt[:, t, :],
                                start=st,
                                stop=sp,
                            )
                            nc.tensor.matmul(
                                vps0[:],
                                xt[:, t, i0 * DK : (i0 + 1) * DK],
                                wv_sb[:, t * 2 * DK : (t + 1) * 2 * DK],
                                start=st,
                                stop=sp,
                            )
                            nc.tensor.matmul(
                                kps[:],
                                wk_sb[:, t * 2 * DK + h * DK : t * 2 * DK + (h + 1) * DK],
                                xt[:, t, :],
                                start=st,
                                stop=sp,
                            )
                            nc.tensor.matmul(
                                vps1[:],
                                xt[:, t, (i0 + 1) * DK : (i0 + 2) * DK],
                                wv_sb[:, t * 2 * DK : (t + 1) * 2 * DK],
                                start=st,
                                stop=sp,
                            )
                        nc.scalar.add(
                            qt_sb[:, h * S + b * CW : h * S + (b + 1) * CW],
                            qps[:],
                            bqk_sb[:, h : h + 1],
                        )
                        nc.scalar.add(
                            kt_sb[:, h * S + b * CW : h * S + (b + 1) * CW],
                            kps[:],
                            bqk_sb[:, 2 + h : 3 + h],
                        )
                        u = 4 * b + i0
                        nc.scalar.copy(
                            v_sb[:, u * 2 * DK : (u + 1) * 2 * DK], vps0[:]
                        )
                        nc.scalar.copy(
                            v_sb[:, (u + 1) * 2 * DK : (u + 2) * 2 * DK], vps1[:]
                        )

            # wo loads during phase 2, lives until the end (LIFO pool stack)
            with tc.tile_pool(name="wop", bufs=1) as wop:
                wo_sb = wop.tile([P, HPC * D], R)
                nc.sync.dma_start(out=wo_sb, in_=wo[:])

                # ---- phase 2: causal attention, scores transposed [k, q]
                with (
                    tc.tile_pool(name="scps", bufs=2, space="PSUM") as scps,
                    tc.tile_pool(name="smps", bufs=2, space="PSUM") as smps,
                    tc.tile_pool(name="ops", bufs=2, space="PSUM") as ops,
                    tc.tile_pool(name="bps", bufs=1, space="PSUM") as bps,
                    tc.tile_pool(name="ptp", bufs=4) as ptp,
                    tc.tile_pool(name="rsp", bufs=2) as rsp,
                    tc.tile_pool(name="bcp", bufs=2) as bcp,
                ):
                    for h in range(HPC):
                        for c in range(C):
                            jmax = 4 * c + 3
                            sum_ps = smps.tile([1, CW], F, name="sum_ps")
                            o_ps = ops.tile([P, CW], F, name="o_ps")
                            for j in range(jmax + 1):
                                t = j - 4 * c
                                lo = P * t if t >= 0 else 0
                                st = j == 0
                                sp = j == jmax
                                sc = scps.tile([P, CW], F, name="sc")
                                nc.tensor.matmul(
                                    sc[:, lo:],
                                    kt_sb[:, h * S + j * P : h * S + (j + 1) * P],
                                    qt_sb[:, h * S + c * CW + lo : h * S + (c + 1) * CW],
                                    start=True,
                                    stop=True,
                                )
                                if t >= 0:
                                    nc.vector.tensor_add(
                                        sc[:, lo : lo + P],
                                        sc[:, lo : lo + P],
                                        mask_sb[:],
                                    )
                                pt = ptp.tile([P, CW], R, name="pt")
                                nc.scalar.activation(
                                    pt[:, lo:], sc[:, lo:], Exp, scale=SCALE
                                )
                                nc.tensor.matmul(
                                    sum_ps[:, lo:],
                                    ones_col[:],
                                    pt[:, lo:],
                                    start=st,
                                    stop=sp,
                                )
                                nc.tensor.matmul(
                                    o_ps[:, lo:],
                                    v_sb[:, j * 2 * DK + h * DK : j * 2 * DK + (h + 1) * DK],
                                    pt[:, lo:],
                                    start=st,
                                    stop=sp,
                                )
                            rs = rsp.tile([1, CW], R, name="rs")
                            with nc.allow_low_precision(reason="fp32r bcast feed"):
                                nc.vector.reciprocal(rs[:], sum_ps[:])
                            b_ps = bps.tile([P, CW], F, name="b_ps")
                            nc.tensor.matmul(
                                b_ps[:], ones_row[:], rs[:], start=True, stop=True
                            )
                            bc = bcp.tile([P, CW], F, name="bc")
                            nc.scalar.copy(bc[:], b_ps[:])
                            nc.vector.tensor_mul(
                                attnT_sb[:, h * S + c * CW : h * S + (c + 1) * CW],
                                o_ps[:],
                                bc[:],
                            )

                # ---- phase 3: O-projection, partial over this core's heads
                with (
                    tc.tile_pool(name="o3ps", bufs=4, space="PSUM") as o3ps,
                    tc.tile_pool(name="outp", bufs=4) as outp,
                ):
                    for u in range(NT):
                        for e in range(C):
                            o3 = o3ps.tile([P, CW], F, name="o3")
                            for h in range(HPC):
                                nc.tensor.matmul(
                                    o3[:],
                                    attnT_sb[:, h * S + u * P : h * S + (u + 1) * P],
                                    wo_sb[:, h * D + e * CW : h * D + (e + 1) * CW],
                                    start=(h == 0),
                                    stop=(h == HPC - 1),
                                )
                            ot = outp.tile([P, CW], F, name="ot")
                            if (u * C + e) % 2 == 0:
                                nc.scalar.copy(ot[:], o3[:])
                            else:
                                nc.vector.tensor_copy(ot[:], o3[:])
                            nc.sync.dma_start(
                                out=out[u * P : (u + 1) * P, e * CW : (e + 1) * CW],
                                in_=ot[:],
                            )

    nc.compile()
    return nc


def _tile_weight_cols(w_slice: np.ndarray) -> np.ndarray:
    """[2048, 256] -> [128, 16*256] with block t = rows [128t, 128t+128)."""
    return np.ascontiguousarray(
        w_slice.reshape(NT, P, 2 * DK).transpose(1, 0, 2).reshape(P, NT * 2 * DK)
    )


def _make_masks() -> np.ndarray:
    """[128,128] additive causal triangle: 0 where p <= f, -1e30 where p > f."""
    p = np.arange(P)[:, None]
    f = np.arange(P)[None, :]
    return np.where(p <= f, 0.0, NEG).astype(np.float32)


def kernel(x, Wq, bq, Wk, bk, Wv, bv, Wo, bo):
    global _NC, last_exec_time_ns, _last_in_maps

    x = np.asarray(x, dtype=np.float32)
    Wq = np.asarray(Wq, dtype=np.float32)
    Wk = np.asarray(Wk, dtype=np.float32)
    Wv = np.asarray(Wv, dtype=np.float32)
    Wo = np.asarray(Wo, dtype=np.float32)
    bq = np.asarray(bq, dtype=np.float32)
    bk = np.asarray(bk, dtype=np.float32)
    bv = np.asarray(bv, dtype=np.float32)
    bo = np.asarray(bo, dtype=np.float32)

    if _NC is None:
        _NC = build()

    xT = np.ascontiguousarray(x[0].T)
    masks = _make_masks()

    in_maps = []
    for i in range(N_CORES):
        cs = slice(2 * DK * i, 2 * DK * (i + 1))
        bqk_i = np.stack(
            [
                bq[2 * DK * i : 2 * DK * i + DK],
                bq[2 * DK * i + DK : 2 * DK * (i + 1)],
                bk[2 * DK * i : 2 * DK * i + DK],
                bk[2 * DK * i + DK : 2 * DK * (i + 1)],
            ],
            axis=1,
        ).astype(np.float32)
        wo_i = np.ascontiguousarray(
            Wo[cs, :].reshape(HPC, P, D).transpose(1, 0, 2).reshape(P, HPC * D)
        )
        in_maps.append(
            {
                "xT": xT,
                "wq": _tile_weight_cols(Wq[:, cs]),
                "wk": _tile_weight_cols(Wk[:, cs]),
                "wv": _tile_weight_cols(Wv[:, cs]),
                "wo": wo_i,
                "bqk": bqk_i,
                "masks": masks,
                "onesc": np.ones((P, 1), np.float32),
                "onesr": np.ones((1, P), np.float32),
            }
        )

    _last_in_maps = in_maps
    trace = bool(int(os.environ.get("BASS_TRACE", "0") or "0"))
    if trace:
        try:
            import ntff_shim

            ntff_shim.install()
        except Exception:
            pass

    res = run_bass_kernel_spmd(
        _NC, in_maps, core_ids=list(range(N_CORES)), trace=trace
    )
    last_exec_time_ns = res.exec_time_ns

    acc = np.zeros((S, D), dtype=np.float64)
    for r_ in res.results:
        acc += r_["out"].astype(np.float64)
    # bv/bo fold: softmax rows sum to 1 => attn @ (V+bv) @ Wo + bo adds bv@Wo + bo
    acc += bv.astype(np.float64) @ Wo.astype(np.float64) + bo.astype(np.float64)
    return acc.astype(np.float32).reshape(1, S, D)


# revision 9
# speedup vs baseline: 1.1109x; 1.0425x over previous
"""TRN2 Bass kernel for nn_MultiHeadAttention_26156350832790.

Multi-head attention: B=1, S=2048, D=2048, H=16 heads (dk=128), causal mask,
fp32 I/O.  Sharded tensor-parallel over 8 NeuronCores: 2 heads per core.

Per-core dataflow (all matmuls in float32r at full PE rate):
  phase 1: Q^T/K^T [dk, S] and V [S, dk] projections, streaming x^T in
           512-column blocks; Q/K (N=512) interleaved with V (N=256) so
           LDWEIGHTS pipelines under the streams
  phase 2: flash-style attention per (head, 512-wide q-chunk), scores kept
           TRANSPOSED [k, q] so softmax sums come from a ones-matmul and the
           PV matmul needs no P transpose; causal handled by slicing the
           q-range per diagonal k-tile plus one [128,128] triangle mask;
           normalization applied to the accumulated output via a broadcast
           reciprocal multiply
  phase 3: O-projection (contraction over the core's 256 head-dims),
           producing a partial [S, D] summed across cores on the host

Host side: x is pre-transposed, weights pre-tiled into SBUF-friendly
layouts; bq/bk applied in-kernel at Q/K evacuation, bv/bo folded into a
host-side row-vector add (softmax rows sum to 1, so P @ (V + bv) ==
P @ V + bv exactly).
"""

import math
import os
import sys

if "/opt/trn_rl_repo" not in sys.path:
    sys.path.insert(0, "/opt/trn_rl_repo")

import numpy as np

import concourse.bacc as bacc
import concourse.tile as tile
from concourse import mybir
from concourse.bass_utils import run_bass_kernel_spmd

P = 128          # partitions
S = 2048         # sequence
D = 2048         # model dim
NT = 16          # 128-row tiles in S or D
HPC = 2          # heads per core
DK = 128         # head dim
C = 4            # 512-wide chunks
CW = 512         # chunk width
N_CORES = 8
SCALE = 1.0 / math.sqrt(DK)
NEG = -1.0e30

F = mybir.dt.float32
R = mybir.dt.float32r

_NC = None
last_exec_time_ns = None
_last_in_maps = None


def build():
    nc = bacc.Bacc(None)

    xT = nc.dram_tensor("xT", [D, S], R, kind="ExternalInput")
    wq = nc.dram_tensor("wq", [P, NT * 2 * DK], R, kind="ExternalInput")
    wk = nc.dram_tensor("wk", [P, NT * 2 * DK], R, kind="ExternalInput")
    wv = nc.dram_tensor("wv", [P, NT * 2 * DK], R, kind="ExternalInput")
    wo = nc.dram_tensor("wo", [P, HPC * D], R, kind="ExternalInput")
    bqk = nc.dram_tensor("bqk", [P, 4], F, kind="ExternalInput")
    masks = nc.dram_tensor("masks", [P, P], F, kind="ExternalInput")
    onesc = nc.dram_tensor("onesc", [P, 1], R, kind="ExternalInput")
    onesr = nc.dram_tensor("onesr", [1, P], R, kind="ExternalInput")
    out = nc.dram_tensor("out", [S, D], F, kind="ExternalOutput")

    Exp = mybir.ActivationFunctionType.Exp

    with tile.TileContext(nc) as tc:
        with (
            tc.tile_pool(name="consts", bufs=1) as consts,
            tc.tile_pool(name="persist", bufs=1) as persist,
        ):
            bqk_sb = consts.tile([P, 4], F)
            mask_sb = consts.tile([P, P], F)
            ones_col = consts.tile([P, 1], R)
            ones_row = consts.tile([1, P], R)
            nc.sync.dma_start(out=bqk_sb, in_=bqk[:])
            nc.sync.dma_start(out=mask_sb, in_=masks[:])
            nc.sync.dma_start(out=ones_col, in_=onesc[:])
            nc.sync.dma_start(out=ones_row, in_=onesr[:])

            # persistent activations
            qt_sb = persist.tile([P, HPC * S], R)      # Q^T per head [dk, S]
            kt_sb = persist.tile([P, HPC * S], R)      # K^T per head [dk, S]
            v_sb = persist.tile([P, NT * 2 * DK], R)   # V s-tiles [k, 2*dk]
            attnT_sb = persist.tile([P, HPC * S], R)   # attn^T per head [dk, S]

            # ---- phase 1: QKV projections, streaming x^T in 512-col blocks
            xT_tiled = xT.rearrange("(t p) s -> p t s", p=P)
            with (
                tc.tile_pool(name="wqkv", bufs=1) as wqkv,
                tc.tile_pool(name="xin", bufs=2) as xin,
                tc.tile_pool(name="p1ps", bufs=2, space="PSUM") as p1ps,
            ):
                wq_sb = wqkv.tile([P, NT * 2 * DK], R)
                wk_sb = wqkv.tile([P, NT * 2 * DK], R)
                wv_sb = wqkv.tile([P, NT * 2 * DK], R)
                nc.sync.dma_start(out=wq_sb, in_=wq[:])
                nc.sync.dma_start(out=wk_sb, in_=wk[:])
                nc.sync.dma_start(out=wv_sb, in_=wv[:])

                for b in range(C):
                    xt = xin.tile([P, NT, CW], R, name="xt")
                    nc.sync.dma_start(
                        out=xt, in_=xT_tiled[:, :, b * CW : (b + 1) * CW]
                    )
                    for h in range(HPC):
                        qps = p1ps.tile([P, CW], F, name="qps")
                        kps = p1ps.tile([P, CW], F, name="kps")
                        vps0 = p1ps.tile([P, 2 * DK], F, name="vps0")
                        vps1 = p1ps.tile([P, 2 * DK], F, name="vps1")
                        i0 = 2 * h
                        for t in range(NT):
                            st = t == 0
                            sp = t == NT - 1
                            nc.tensor.matmul(
                                qps[:],
                                wq_sb[:, t * 2 * DK + h * DK : t * 2 * DK + (h + 1) * DK],
                                xt[:, t, :],
                                start=st,
                                stop=sp,
                            )
                            nc.tensor.matmul(
                                vps0[:],
                                xt[:, t, i0 * DK : (i0 + 1) * DK],
                                wv_sb[:, t * 2 * DK : (t + 1) * 2 * DK],
                                start=st,
                                stop=sp,
                            )
                            nc.tensor.matmul(
                                kps[:],
                                wk_sb[:, t * 2 * DK + h * DK : t * 2 * DK + (h + 1) * DK],
                                xt[:, t, :],
                                start=st,
                                stop=sp,
                            )
                            nc.tensor.matmul(
                                vps1[:],
                                xt[:, t, (i0 + 1) * DK : (i0 + 2) * DK],
                                wv_sb[:, t * 2 * DK : (t + 1) * 2 * DK],
                                start=st,
                                stop=sp,
                            )
                        nc.scalar.add(
                            qt_sb[:, h * S + b * CW : h * S + (b + 1) * CW],
                            qps[:],
                            bqk_sb[:, h : h + 1],
                        )
                        nc.scalar.add(
                            kt_sb[:, h * S + b * CW : h * S + (b + 1) * CW],
                            kps[:],
                            bqk_sb[:, 2 + h : 3 + h],
                        )
                        u = 4 * b + i0
                        nc.scalar.copy(
                            v_sb[:, u * 2 * DK : (u + 1) * 2 * DK], vps0[:]
                        )
                        nc.scalar.copy(
                            v_sb[:, (u + 1) * 2 * DK : (u + 2) * 2 * DK], vps1[:]
                        )

            # wo loads during phase 2, lives until the end (LIFO pool stack)
            with tc.tile_pool(name="wop", bufs=1) as wop:
                wo_sb = wop.tile([P, HPC * D], R)
                nc.sync.dma_start(out=wo_sb, in_=wo[:])

                # ---- phases 2+3 interleaved per 512-chunk: causal attention
                # (scores transposed [k, q]) then the O-projection for the
                # chunk's s-tiles, so output DMA spreads across the run.
                with (
                    tc.tile_pool(name="ps23", bufs=1, space="PSUM") as ps23,
                    tc.tile_pool(name="ptp", bufs=6) as ptp,
                    tc.tile_pool(name="ssp", bufs=2) as ssp,
                    tc.tile_pool(name="bcp", bufs=2) as bcp,
                    tc.tile_pool(name="outp", bufs=4) as outp,
                ):
                    for c in range(C):
                        for h in range(HPC):
                            jmax = 4 * c + 3
                            sum_ps = ps23.tile([1, CW], F, name="sum_ps", tag="B", bufs=2)
                            o_ps = ps23.tile([P, CW], F, name="o_ps", tag="Cc", bufs=2)
                            lag = None  # (pt, lo, start, stop) pending PV/sums
                            for j in range(jmax + 1):
                                t = j - 4 * c
                                lo = P * t if t >= 0 else 0
                                sc = ps23.tile([P, CW], F, name="sc", tag="A", bufs=3)
                                nc.tensor.matmul(
                                    sc[:, lo:],
                                    kt_sb[:, h * S + j * P : h * S + (j + 1) * P],
                                    qt_sb[:, h * S + c * CW + lo : h * S + (c + 1) * CW],
                                    start=True,
                                    stop=True,
                                )
                                if t >= 0:
                                    nc.vector.tensor_add(
                                        sc[:, lo : lo + P],
                                        sc[:, lo : lo + P],
                                        mask_sb[:],
                                    )
                                pt = ptp.tile([P, CW], R, name="pt")
                                nc.scalar.activation(
                                    pt[:, lo:], sc[:, lo:], Exp, scale=SCALE
                                )
                                if lag is not None:
                                    lpt, llo, lst, lsp, lj = lag
                                    nc.tensor.matmul(
                                        o_ps[:, llo:],
                                        v_sb[:, lj * 2 * DK + h * DK : lj * 2 * DK + (h + 1) * DK],
                                        lpt[:, llo:],
                                        start=lst,
                                        stop=lsp,
                                    )
                                    nc.tensor.matmul(
                                        sum_ps[:, llo:],
                                        ones_col[:],
                                        lpt[:, llo:],
                                        start=lst,
                                        stop=lsp,
                                    )
                                lag = (pt, lo, j == 0, j == jmax, j)
                            lpt, llo, lst, lsp, lj = lag
                            nc.tensor.matmul(
                                o_ps[:, llo:],
                                v_sb[:, lj * 2 * DK + h * DK : lj * 2 * DK + (h + 1) * DK],
                                lpt[:, llo:],
                                start=lst,
                                stop=lsp,
                            )
                            nc.tensor.matmul(
                                sum_ps[:, llo:],
                                ones_col[:],
                                lpt[:, llo:],
                                start=lst,
                                stop=lsp,
                            )
                            ss = ssp.tile([1, CW], R, name="ss")
                            with nc.allow_low_precision(reason="fp32r bcast feed"):
                                nc.scalar.copy(ss[:], sum_ps[:])
                            b_ps = ps23.tile([P, CW], F, name="b_ps", tag="D", bufs=1)
                            nc.tensor.matmul(
                                b_ps[:], ones_row[:], ss[:], start=True, stop=True
                            )
                            bc = bcp.tile([P, CW], F, name="bc")
                            nc.vector.reciprocal(bc[:], b_ps[:])
                            nc.vector.tensor_mul(
                                attnT_sb[:, h * S + c * CW : h * S + (c + 1) * CW],
                                o_ps[:],
                                bc[:],
                            )

                        # O-projection for this chunk's four s-tiles
                        for u in range(4 * c, 4 * c + 4):
                            for e in range(C):
                                o3 = ps23.tile([P, CW], F, name="o3", tag="A", bufs=3)
                                for h in range(HPC):
                                    nc.tensor.matmul(
                                        o3[:],
                                        attnT_sb[:, h * S + u * P : h * S + (u + 1) * P],
                                        wo_sb[:, h * D + e * CW : h * D + (e + 1) * CW],
                                        start=(h == 0),
                                        stop=(h == HPC - 1),
                                    )
                                ot = outp.tile([P, CW], F, name="ot")
                                if (u * C + e) % 2 == 0:
                                    nc.scalar.copy(ot[:], o3[:])
                                else:
                                    nc.vector.tensor_copy(ot[:], o3[:])
                                nc.sync.dma_start(
                                    out=out[u * P : (u + 1) * P, e * CW : (e + 1) * CW],
                                    in_=ot[:],
                                )

    nc.compile()
    return nc


def _tile_weight_cols(w_slice: np.ndarray) -> np.ndarray:
    """[2048, 256] -> [128, 16*256] with block t = rows [128t, 128t+128)."""
    return np.ascontiguousarray(
        w_slice.reshape(NT, P, 2 * DK).transpose(1, 0, 2).reshape(P, NT * 2 * DK)
    )


def _make_masks() -> np.ndarray:
    """[128,128] additive causal triangle: 0 where p <= f, -1e30 where p > f."""
    p = np.arange(P)[:, None]
    f = np.arange(P)[None, :]
    return np.where(p <= f, 0.0, NEG).astype(np.float32)


def kernel(x, Wq, bq, Wk, bk, Wv, bv, Wo, bo):
    global _NC, last_exec_time_ns, _last_in_maps

    x = np.asarray(x, dtype=np.float32)
    Wq = np.asarray(Wq, dtype=np.float32)
    Wk = np.asarray(Wk, dtype=np.float32)
    Wv = np.asarray(Wv, dtype=np.float32)
    Wo = np.asarray(Wo, dtype=np.float32)
    bq = np.asarray(bq, dtype=np.float32)
    bk = np.asarray(bk, dtype=np.float32)
    bv = np.asarray(bv, dtype=np.float32)
    bo = np.asarray(bo, dtype=np.float32)

    if _NC is None:
        _NC = build()

    xT = np.ascontiguousarray(x[0].T)
    masks = _make_masks()

    in_maps = []
    for i in range(N_CORES):
        cs = slice(2 * DK * i, 2 * DK * (i + 1))
        bqk_i = np.stack(
            [
                bq[2 * DK * i : 2 * DK * i + DK],
                bq[2 * DK * i + DK : 2 * DK * (i + 1)],
                bk[2 * DK * i : 2 * DK * i + DK],
                bk[2 * DK * i + DK : 2 * DK * (i + 1)],
            ],
            axis=1,
        ).astype(np.float32)
        wo_i = np.ascontiguousarray(
            Wo[cs, :].reshape(HPC, P, D).transpose(1, 0, 2).reshape(P, HPC * D)
        )
        in_maps.append(
            {
                "xT": xT,
                "wq": _tile_weight_cols(Wq[:, cs]),
                "wk": _tile_weight_cols(Wk[:, cs]),
                "wv": _tile_weight_cols(Wv[:, cs]),
                "wo": wo_i,
                "bqk": bqk_i,
                "masks": masks,
                "onesc": np.ones((P, 1), np.float32),
                "onesr": np.ones((1, P), np.float32),
            }
        )

    _last_in_maps = in_maps
    trace = bool(int(os.environ.get("BASS_TRACE", "0") or "0"))
    if trace:
        try:
            import ntff_shim

            ntff_shim.install()
        except Exception:
            pass

    res = run_bass_kernel_spmd(
        _NC, in_maps, core_ids=list(range(N_CORES)), trace=trace
    )
    last_exec_time_ns = res.exec_time_ns

    acc = np.zeros((S, D), dtype=np.float64)
    for r_ in res.results:
        acc += r_["out"].astype(np.float64)
    # bv/bo fold: softmax rows sum to 1 => attn @ (V+bv) @ Wo + bo adds bv@Wo + bo
    acc += bv.astype(np.float64) @ Wo.astype(np.float64) + bo.astype(np.float64)
    return acc.astype(np.float32).reshape(1, S, D)


# revision 10
# speedup vs baseline: 1.1225x; 1.0105x over previous
"""TRN2 Bass kernel for nn_MultiHeadAttention_26156350832790.

Multi-head attention: B=1, S=2048, D=2048, H=16 heads (dk=128), causal mask,
fp32 I/O.  Sharded tensor-parallel over 8 NeuronCores: 2 heads per core.

Per-core dataflow (all matmuls in float32r at full PE rate):
  phase 1: Q^T/K^T [dk, S] and V [S, dk] projections, streaming x^T in
           512-column blocks; Q/K (N=512) interleaved with V (N=256) so
           LDWEIGHTS pipelines under the streams
  phase 2: flash-style attention per (head, 512-wide q-chunk), scores kept
           TRANSPOSED [k, q] so softmax sums come from a ones-matmul and the
           PV matmul needs no P transpose; causal handled by slicing the
           q-range per diagonal k-tile plus one [128,128] triangle mask;
           normalization applied to the accumulated output via a broadcast
           reciprocal multiply
  phase 3: O-projection (contraction over the core's 256 head-dims),
           producing a partial [S, D] summed across cores on the host

Host side: x is pre-transposed, weights pre-tiled into SBUF-friendly
layouts; bq/bk applied in-kernel at Q/K evacuation, bv/bo folded into a
host-side row-vector add (softmax rows sum to 1, so P @ (V + bv) ==
P @ V + bv exactly).
"""

import math
import os
import sys

if "/opt/trn_rl_repo" not in sys.path:
    sys.path.insert(0, "/opt/trn_rl_repo")

import numpy as np

import concourse.bacc as bacc
import concourse.tile as tile
from concourse import mybir
from concourse.bass_utils import run_bass_kernel_spmd

P = 128          # partitions
S = 2048         # sequence
D = 2048         # model dim
NT = 16          # 128-row tiles in S or D
HPC = 2          # heads per core
DK = 128         # head dim
C = 4            # 512-wide chunks
CW = 512         # chunk width
N_CORES = 8
SCALE = 1.0 / math.sqrt(DK)
NEG = -1.0e30

F = mybir.dt.float32
R = mybir.dt.float32r

_NC = None
last_exec_time_ns = None
_last_in_maps = None


def build():
    nc = bacc.Bacc(None)

    xT = nc.dram_tensor("xT", [D, S], R, kind="ExternalInput")
    wq = nc.dram_tensor("wq", [P, NT * 2 * DK], R, kind="ExternalInput")
    wk = nc.dram_tensor("wk", [P, NT * 2 * DK], R, kind="ExternalInput")
    wv = nc.dram_tensor("wv", [P, NT * 2 * DK], R, kind="ExternalInput")
    wo = nc.dram_tensor("wo", [P, HPC * D], R, kind="ExternalInput")
    bqk = nc.dram_tensor("bqk", [P, 4], F, kind="ExternalInput")
    masks = nc.dram_tensor("masks", [P, P], F, kind="ExternalInput")
    onesc = nc.dram_tensor("onesc", [P, 1], R, kind="ExternalInput")
    onesr = nc.dram_tensor("onesr", [1, P], R, kind="ExternalInput")
    out = nc.dram_tensor("out", [S, D], F, kind="ExternalOutput")

    Exp = mybir.ActivationFunctionType.Exp

    with tile.TileContext(nc) as tc:
        with (
            tc.tile_pool(name="consts", bufs=1) as consts,
            tc.tile_pool(name="persist", bufs=1) as persist,
        ):
            bqk_sb = consts.tile([P, 4], F)
            mask_sb = consts.tile([P, P], F)
            ones_col = consts.tile([P, 1], R)
            ones_row = consts.tile([1, P], R)
            nc.sync.dma_start(out=bqk_sb, in_=bqk[:])
            nc.sync.dma_start(out=mask_sb, in_=masks[:])
            nc.sync.dma_start(out=ones_col, in_=onesc[:])
            nc.sync.dma_start(out=ones_row, in_=onesr[:])

            # persistent activations
            qt_sb = persist.tile([P, HPC * S], R)      # Q^T per head [dk, S]
            kt_sb = persist.tile([P, HPC * S], R)      # K^T per head [dk, S]
            v_sb = persist.tile([P, NT * 2 * DK], R)   # V s-tiles [k, 2*dk]
            attnT_sb = persist.tile([P, HPC * S], R)   # attn^T per head [dk, S]

            # ---- phase 1: QKV projections, streaming x^T in 512-col blocks
            xT_tiled = xT.rearrange("(t p) s -> p t s", p=P)
            with (
                tc.tile_pool(name="wqkv", bufs=1) as wqkv,
                tc.tile_pool(name="xin", bufs=2) as xin,
                tc.tile_pool(name="p1ps", bufs=2, space="PSUM") as p1ps,
            ):
                wq_sb = wqkv.tile([P, NT * 2 * DK], R)
                wk_sb = wqkv.tile([P, NT * 2 * DK], R)
                wv_sb = wqkv.tile([P, NT * 2 * DK], R)
                xts = []
                for b in range(C):
                    xtb = xin.tile([P, NT, CW], R, name=f"xt{b}", tag="xt")
                    xts.append(xtb)
                # per-t slices so the first matmuls start ~35us earlier
                for t in range(NT):
                    ws = slice(t * 2 * DK, (t + 1) * 2 * DK)
                    nc.sync.dma_start(out=wq_sb[:, ws], in_=wq[:, ws])
                    nc.sync.dma_start(out=wk_sb[:, ws], in_=wk[:, ws])
                    nc.sync.dma_start(out=wv_sb[:, ws], in_=wv[:, ws])
                    nc.sync.dma_start(
                        out=xts[0][:, t, :], in_=xT_tiled[:, t, 0:CW]
                    )

                for b in range(C):
                    xt = xts[b]
                    if b > 0:
                        nc.sync.dma_start(
                            out=xt, in_=xT_tiled[:, :, b * CW : (b + 1) * CW]
                        )
                    for h in range(HPC):
                        qps = p1ps.tile([P, CW], F, name="qps")
                        kps = p1ps.tile([P, CW], F, name="kps")
                        vps0 = p1ps.tile([P, 2 * DK], F, name="vps0")
                        vps1 = p1ps.tile([P, 2 * DK], F, name="vps1")
                        i0 = 2 * h
                        for t in range(NT):
                            st = t == 0
                            sp = t == NT - 1
                            nc.tensor.matmul(
                                qps[:],
                                wq_sb[:, t * 2 * DK + h * DK : t * 2 * DK + (h + 1) * DK],
                                xt[:, t, :],
                                start=st,
                                stop=sp,
                            )
                            nc.tensor.matmul(
                                vps0[:],
                                xt[:, t, i0 * DK : (i0 + 1) * DK],
                                wv_sb[:, t * 2 * DK : (t + 1) * 2 * DK],
                                start=st,
                                stop=sp,
                            )
                            nc.tensor.matmul(
                                kps[:],
                                wk_sb[:, t * 2 * DK + h * DK : t * 2 * DK + (h + 1) * DK],
                                xt[:, t, :],
                                start=st,
                                stop=sp,
                            )
                            nc.tensor.matmul(
                                vps1[:],
                                xt[:, t, (i0 + 1) * DK : (i0 + 2) * DK],
                                wv_sb[:, t * 2 * DK : (t + 1) * 2 * DK],
                                start=st,
                                stop=sp,
                            )
                        nc.scalar.add(
                            qt_sb[:, h * S + b * CW : h * S + (b + 1) * CW],
                            qps[:],
                            bqk_sb[:, h : h + 1],
                        )
                        nc.scalar.add(
                            kt_sb[:, h * S + b * CW : h * S + (b + 1) * CW],
                            kps[:],
                            bqk_sb[:, 2 + h : 3 + h],
                        )
                        u = 4 * b + i0
                        nc.scalar.copy(
                            v_sb[:, u * 2 * DK : (u + 1) * 2 * DK], vps0[:]
                        )
                        nc.scalar.copy(
                            v_sb[:, (u + 1) * 2 * DK : (u + 2) * 2 * DK], vps1[:]
                        )

            # wo loads during phase 2, lives until the end (LIFO pool stack)
            with tc.tile_pool(name="wop", bufs=1) as wop:
                wo_sb = wop.tile([P, HPC * D], R)
                nc.sync.dma_start(out=wo_sb, in_=wo[:])

                # ---- phases 2+3 interleaved per 512-chunk: causal attention
                # (scores transposed [k, q]) then the O-projection for the
                # chunk's s-tiles, so output DMA spreads across the run.
                with (
                    tc.tile_pool(name="ps23", bufs=1, space="PSUM") as ps23,
                    tc.tile_pool(name="ptp", bufs=6) as ptp,
                    tc.tile_pool(name="ssp", bufs=2) as ssp,
                    tc.tile_pool(name="bcp", bufs=2) as bcp,
                    tc.tile_pool(name="outp", bufs=4) as outp,
                ):
                    for c in range(C):
                        for h in range(HPC):
                            jmax = 4 * c + 3
                            sum_ps = ps23.tile([1, CW], F, name="sum_ps", tag="B", bufs=2)
                            o_ps = ps23.tile([P, CW], F, name="o_ps", tag="Cc", bufs=2)
                            lag = None  # (pt, lo, start, stop) pending PV/sums
                            for j in range(jmax + 1):
                                t = j - 4 * c
                                lo = P * t if t >= 0 else 0
                                sc = ps23.tile([P, CW], F, name="sc", tag="A", bufs=3)
                                nc.tensor.matmul(
                                    sc[:, lo:],
                                    kt_sb[:, h * S + j * P : h * S + (j + 1) * P],
                                    qt_sb[:, h * S + c * CW + lo : h * S + (c + 1) * CW],
                                    start=True,
                                    stop=True,
                                )
                                if t >= 0:
                                    nc.vector.tensor_add(
                                        sc[:, lo : lo + P],
                                        sc[:, lo : lo + P],
                                        mask_sb[:],
                                    )
                                pt = ptp.tile([P, CW], R, name="pt")
                                nc.scalar.activation(
                                    pt[:, lo:], sc[:, lo:], Exp, scale=SCALE
                                )
                                if lag is not None:
                                    lpt, llo, lst, lsp, lj = lag
                                    nc.tensor.matmul(
                                        o_ps[:, llo:],
                                        v_sb[:, lj * 2 * DK + h * DK : lj * 2 * DK + (h + 1) * DK],
                                        lpt[:, llo:],
                                        start=lst,
                                        stop=lsp,
                                    )
                                    nc.tensor.matmul(
                                        sum_ps[:, llo:],
                                        ones_col[:],
                                        lpt[:, llo:],
                                        start=lst,
                                        stop=lsp,
                                    )
                                lag = (pt, lo, j == 0, j == jmax, j)
                            lpt, llo, lst, lsp, lj = lag
                            nc.tensor.matmul(
                                o_ps[:, llo:],
                                v_sb[:, lj * 2 * DK + h * DK : lj * 2 * DK + (h + 1) * DK],
                                lpt[:, llo:],
                                start=lst,
                                stop=lsp,
                            )
                            nc.tensor.matmul(
                                sum_ps[:, llo:],
                                ones_col[:],
                                lpt[:, llo:],
                                start=lst,
                                stop=lsp,
                            )
                            ss = ssp.tile([1, CW], R, name="ss")
                            with nc.allow_low_precision(reason="fp32r bcast feed"):
                                nc.scalar.copy(ss[:], sum_ps[:])
                            b_ps = ps23.tile([P, CW], F, name="b_ps", tag="D", bufs=1)
                            nc.tensor.matmul(
                                b_ps[:], ones_row[:], ss[:], start=True, stop=True
                            )
                            bc = bcp.tile([P, CW], F, name="bc")
                            nc.vector.reciprocal(bc[:], b_ps[:])
                            nc.vector.tensor_mul(
                                attnT_sb[:, h * S + c * CW : h * S + (c + 1) * CW],
                                o_ps[:],
                                bc[:],
                            )

                        # O-projection lags one chunk so its matmuls hide the
                        # normalization-chain latency of the current chunk
                        co = c - 1
                        if co < 0:
                            continue
                        for u in range(4 * co, 4 * co + 4):
                            for e in range(C):
                                o3 = ps23.tile([P, CW], F, name="o3", tag="A", bufs=3)
                                for h in range(HPC):
                                    nc.tensor.matmul(
                                        o3[:],
                                        attnT_sb[:, h * S + u * P : h * S + (u + 1) * P],
                                        wo_sb[:, h * D + e * CW : h * D + (e + 1) * CW],
                                        start=(h == 0),
                                        stop=(h == HPC - 1),
                                    )
                                ot = outp.tile([P, CW], F, name="ot")
                                if (u * C + e) % 2 == 0:
                                    nc.scalar.copy(ot[:], o3[:])
                                else:
                                    nc.vector.tensor_copy(ot[:], o3[:])
                                nc.sync.dma_start(
                                    out=out[u * P : (u + 1) * P, e * CW : (e + 1) * CW],
                                    in_=ot[:],
                                )

                    for u in range(4 * (C - 1), 4 * C):
                        for e in range(C):
                            o3 = ps23.tile([P, CW], F, name="o3", tag="A", bufs=3)
                            for h in range(HPC):
                                nc.tensor.matmul(
                                    o3[:],
                                    attnT_sb[:, h * S + u * P : h * S + (u + 1) * P],
                                    wo_sb[:, h * D + e * CW : h * D + (e + 1) * CW],
                                    start=(h == 0),
                                    stop=(h == HPC - 1),
                                )
                            ot = outp.tile([P, CW], F, name="ot")
                            if (u * C + e) % 2 == 0:
                                nc.scalar.copy(ot[:], o3[:])
                            else:
                                nc.vector.tensor_copy(ot[:], o3[:])
                            nc.sync.dma_start(
                                out=out[u * P : (u + 1) * P, e * CW : (e + 1) * CW],
                                in_=ot[:],
                            )

    nc.compile()
    return nc


def _tile_weight_cols(w_slice: np.ndarray) -> np.ndarray:
    """[2048, 256] -> [128, 16*256] with block t = rows [128t, 128t+128)."""
    return np.ascontiguousarray(
        w_slice.reshape(NT, P, 2 * DK).transpose(1, 0, 2).reshape(P, NT * 2 * DK)
    )


def _make_masks() -> np.ndarray:
    """[128,128] additive causal triangle: 0 where p <= f, -1e30 where p > f."""
    p = np.arange(P)[:, None]
    f = np.arange(P)[None, :]
    return np.where(p <= f, 0.0, NEG).astype(np.float32)


def kernel(x, Wq, bq, Wk, bk, Wv, bv, Wo, bo):
    global _NC, last_exec_time_ns, _last_in_maps

    x = np.asarray(x, dtype=np.float32)
    Wq = np.asarray(Wq, dtype=np.float32)
    Wk = np.asarray(Wk, dtype=np.float32)
    Wv = np.asarray(Wv, dtype=np.float32)
    Wo = np.asarray(Wo, dtype=np.float32)
    bq = np.asarray(bq, dtype=np.float32)
    bk = np.asarray(bk, dtype=np.float32)
    bv = np.asarray(bv, dtype=np.float32)
    bo = np.asarray(bo, dtype=np.float32)

    if _NC is None:
        _NC = build()

    xT = np.ascontiguousarray(x[0].T)
    masks = _make_masks()

    in_maps = []
    for i in range(N_CORES):
        cs = slice(2 * DK * i, 2 * DK * (i + 1))
        bqk_i = np.stack(
            [
                bq[2 * DK * i : 2 * DK * i + DK],
                bq[2 * DK * i + DK : 2 * DK * (i + 1)],
                bk[2 * DK * i : 2 * DK * i + DK],
                bk[2 * DK * i + DK : 2 * DK * (i + 1)],
            ],
            axis=1,
        ).astype(np.float32)
        wo_i = np.ascontiguousarray(
            Wo[cs, :].reshape(HPC, P, D).transpose(1, 0, 2).reshape(P, HPC * D)
        )
        in_maps.append(
            {
                "xT": xT,
                "wq": _tile_weight_cols(Wq[:, cs]),
                "wk": _tile_weight_cols(Wk[:, cs]),
                "wv": _tile_weight_cols(Wv[:, cs]),
                "wo": wo_i,
                "bqk": bqk_i,
                "masks": masks,
                "onesc": np.ones((P, 1), np.float32),
                "onesr": np.ones((1, P), np.float32),
            }
        )

    _last_in_maps = in_maps
    trace = bool(int(os.environ.get("BASS_TRACE", "0") or "0"))
    if trace:
        try:
            import ntff_shim

            ntff_shim.install()
        except Exception:
            pass

    res = run_bass_kernel_spmd(
        _NC, in_maps, core_ids=list(range(N_CORES)), trace=trace
    )
    last_exec_time_ns = res.exec_time_ns

    acc = np.zeros((S, D), dtype=np.float64)
    for r_ in res.results:
        acc += r_["out"].astype(np.float64)
    # bv/bo fold: softmax rows sum to 1 => attn @ (V+bv) @ Wo + bo adds bv@Wo + bo
    acc += bv.astype(np.float64) @ Wo.astype(np.float64) + bo.astype(np.float64)
    return acc.astype(np.float32).reshape(1, S, D)
